# revision 9
# baseline (speedup 1.0000x reference)
"""Trainium2 Bass kernel for nn_Attention_66314295050336.

Sparse (threshold-pruned) multi-head attention:
    qkv  = x @ w_qkv.T + b_qkv          [B,N,3C]
    q,k,v heads (H=6, D=64), attn = softmax(mask(q@k.T * D**-0.5))
    mask: scores < 0.0 -> -10000 before softmax (=> weight 0 in fp32)
    out  = (attn @ v) @ w_proj.T + b_proj

Sharding: pure data-parallel over batch B=8 across the 8 NeuronCores
(one batch per core, no collectives).  Per core everything stays
SBUF-resident; all matmuls are fp16 (1 cyc/row on the PE) with fp32
PSUM accumulation:

  xT[c,n] --(fp16 MM)--> qkT [768,1024] (heads paired on partitions)
                     +-> v   [1024,384] (fp16)
  scoresT[k,q] per head pair via row-packed K=64 matmuls
  ACT exp(scale*s) psum->SBUF fp16; threshold mask on DVE:
    b = (e >= 1)  [tensor_scalar, 4x]; e *= b [tensor_tensor, 2x]
  (e >= 1  <=>  score >= 0, measure-zero edge at exactly 0)
  attn@v: col-packed M=64 pairs, lhsT=v; a parallel ones-matmul gives
  Z (softmax denominator) replicated across 64 partitions, so
  normalization is reciprocal_approx_fast + one tensor_tensor multiply.
  proj: fp16 matmuls from SBUF, copied PSUM->SBUF, DMAed to HBM.

Biases are zeros per the problem spec (fill: zeros); asserted below.
"""

import os
import sys

import numpy as np

for _p in ("/opt/trn_rl_repo", "/root/.axon_site/_ro/trn_rl_repo"):
    if os.path.isdir(_p) and _p not in sys.path:
        sys.path.insert(0, _p)

N = 1024
C = 384
H = 6
D = 64
SCALE = float(D) ** -0.5  # 0.125
NCORES = 8

_CACHE = {}


def _build():
    import concourse.bass as bass
    import concourse.mybir as mybir
    import concourse.tile as tile
    from concourse import bacc
    from contextlib import ExitStack

    F32 = mybir.dt.float32
    F16 = mybir.dt.float16
    MULT = mybir.AluOpType.mult
    IS_GE = mybir.AluOpType.is_ge
    EXP = mybir.ActivationFunctionType.Exp

    nc = bacc.Bacc(
        "TRN2", target_bir_lowering=False, debug=False, enable_asserts=False
    )

    xT_d = nc.dram_tensor("xT", [C, N], F16, kind="ExternalInput")
    wqkvT_d = nc.dram_tensor("wqkvT", [C, 3 * C], F16, kind="ExternalInput")
    wprojT_d = nc.dram_tensor("wprojT", [C, C], F16, kind="ExternalInput")
    out_d = nc.dram_tensor("out", [N, C], F32, kind="ExternalOutput")

    with tile.TileContext(nc) as tc, ExitStack() as ctx:
        const = ctx.enter_context(tc.tile_pool(name="const", bufs=1))
        epool = ctx.enter_context(tc.tile_pool(name="e", bufs=2))
        bpool = ctx.enter_context(tc.tile_pool(name="bn", bufs=4))
        psS = ctx.enter_context(
            tc.tile_pool(name="psS", bufs=2, space=bass.MemorySpace.PSUM)
        )
        psO = ctx.enter_context(
            tc.tile_pool(name="psO", bufs=2, space=bass.MemorySpace.PSUM)
        )

        xT = const.tile([128, 3 * N], F16)  # c-tile ct -> cols [ct*N:(ct+1)*N]
        wqkv = const.tile([128, 3 * 3 * C], F16)  # ct -> cols [ct*1152 ...]
        wproj = const.tile([128, 3 * C], F16)
        qk = const.tile([128, 6 * N], F16)  # q pairs 0..2, k pairs 3..5
        v = const.tile([128, 8 * C], F16)  # n-tile nt -> cols [nt*384 ...]
        ones64 = const.tile([128, 64], F16)
        outT = const.tile([128, 3 * N], F16)  # pair p -> cols [p*N:(p+1)*N]

        for ct in range(3):
            r = slice(ct * 128, (ct + 1) * 128)
            nc.sync.dma_start(xT[:, ct * N : (ct + 1) * N], xT_d[r, :])
            nc.sync.dma_start(
                wqkv[:, ct * 1152 : (ct + 1) * 1152], wqkvT_d[r, :]
            )
            nc.sync.dma_start(wproj[:, ct * C : (ct + 1) * C], wprojT_d[r, :])
        nc.gpsimd.memset(ones64[:], 1.0)

        ps_tags = [("psS", "s"), ("psS", "s"), ("psO", "O"), ("psO", "Zb")]

        def mm_psum(pool_idx, shape):
            pn, tag = ps_tags[pool_idx % 4]
            pool = psS if pn == "psS" else psO
            return pool.tile(shape, F32, tag=tag, name=f"mmps_{pool_idx}")

        # ---------------- helpers: qkv-production groups ------------------
        gi_box = [0]

        def emit_qk_chunk(oc):
            # qkT o-chunk oc -> qk cols [oc*N : (oc+1)*N]
            for nh in range(2):
                ps = mm_psum(gi_box[0], [128, 512])
                gi_box[0] += 1
                for ct in range(3):
                    nc.tensor.matmul(
                        ps[:],
                        wqkv[
                            :, ct * 1152 + oc * 128 : ct * 1152 + (oc + 1) * 128
                        ],
                        xT[:, ct * N + nh * 512 : ct * N + nh * 512 + 512],
                        start=(ct == 0),
                        stop=(ct == 2),
                    )
                dst = qk[:, oc * N + nh * 512 : oc * N + nh * 512 + 512]
                if oc % 2 == 0:
                    nc.scalar.copy(dst, ps[:])
                else:
                    nc.vector.tensor_copy(dst, ps[:])

        # ---------------- v: [1024, 384] fp16 (n on partitions) -----------
        for nt in range(8):
            ps = mm_psum(gi_box[0], [128, 384])
            gi_box[0] += 1
            for ct in range(3):
                nc.tensor.matmul(
                    ps[:],
                    xT[:, ct * N + nt * 128 : ct * N + (nt + 1) * 128],
                    wqkv[:, ct * 1152 + 768 : ct * 1152 + 1152],
                    start=(ct == 0),
                    stop=(ct == 2),
                )
            nc.scalar.copy(v[:, nt * C : (nt + 1) * C], ps[:])

        emit_qk_chunk(0)  # q pair 0
        emit_qk_chunk(3)  # k pair 0

        # ---------------- attention, head pairs, qc-outer -----------------
        # e_pair layout: block bi=(qc*8+kt) -> cols [bi*1024 : bi*1024+1024],
        # block = [h_even 512 | h_odd 512] for that (kt, qc).
        for p in range(3):
            h_ev, h_od = 2 * p, 2 * p + 1
            e_pair = epool.tile([128, 16 * N], F16, tag="e", name=f"e_{p}")
            qT0 = p * N
            kT0 = (3 + p) * N
            if p < 2:  # produce next pair's qT/kT as PE fill-in work
                emit_qk_chunk(p + 1)
                emit_qk_chunk(3 + p + 1)

            for qc in range(2):
                O_ps = psO.tile([128, 512], F32, tag="O", name=f"O_{p}_{qc}")
                Z_ps = psO.tile([128, 512], F32, tag="Zb", name=f"Z_{p}_{qc}")
                for kt in range(8):
                    bi = qc * 8 + kt
                    s = psS.tile([128, 1024], F32, tag="s", name=f"s_{p}_{bi}")
                    # scoresT[k,q] = sum_d kT[d,k]*qT[d,q]; heads row-packed.
                    # Critical section keeps the K=64 row-pair adjacent so the
                    # two matmuls run concurrently on disjoint array halves.
                    with tc.tile_critical():
                        nc.tensor.matmul(
                            s[:, 0:512],
                            qk[0:64, kT0 + kt * 128 : kT0 + (kt + 1) * 128],
                            qk[0:64, qT0 + qc * 512 : qT0 + (qc + 1) * 512],
                            start=True,
                            stop=True,
                        )
                        nc.tensor.matmul(
                            s[:, 512:1024],
                            qk[64:128, kT0 + kt * 128 : kT0 + (kt + 1) * 128],
                            qk[64:128, qT0 + qc * 512 : qT0 + (qc + 1) * 512],
                            start=True,
                            stop=True,
                        )
                    eb = e_pair[:, bi * 1024 : (bi + 1) * 1024]
                    nc.scalar.activation(eb, s[:], EXP, scale=SCALE)
                    if kt % 2 == 0:
                        continue
                    # threshold mask over two kt blocks at once:
                    # b = (e>=1) [TS 4x]; e *= b [TT 2x]
                    e2 = e_pair[:, (bi - 1) * 1024 : (bi + 1) * 1024]
                    b = bpool.tile(
                        [128, 2048], F16, tag="b", name=f"b_{p}_{bi}"
                    )
                    nc.vector.tensor_scalar(b[:], e2, 1.0, None, IS_GE)
                    nc.vector.tensor_mul(e2, e2, b[:])
                    for kb in (kt - 1, kt):
                        bj = qc * 8 + kb
                        rev = e_pair[:, bj * 1024 : bj * 1024 + 512]
                        rod = e_pair[:, bj * 1024 + 512 : bj * 1024 + 1024]
                        vev = v[:, kb * C + h_ev * 64 : kb * C + h_ev * 64 + 64]
                        vod = v[:, kb * C + h_od * 64 : kb * C + h_od * 64 + 64]
                        st, sp = (kb == 0), (kb == 7)
                        # outT_h[d,q] accumulation, two heads col-packed
                        nc.tensor.matmul(
                            O_ps[0:64, :], vev, rev, start=st, stop=sp,
                            tile_position=(0, 0), skip_group_check=True,
                        )
                        nc.tensor.matmul(
                            O_ps[64:128, :], vod, rod, start=st, stop=sp,
                            tile_position=(0, 64), skip_group_check=True,
                        )
                        # Z_h[q] (replicated x64): ones-matmul, same rhs
                        nc.tensor.matmul(
                            Z_ps[0:64, :], ones64[:, 0:64], rev, start=st,
                            stop=sp, tile_position=(0, 0),
                            skip_group_check=True,
                        )
                        nc.tensor.matmul(
                            Z_ps[64:128, :], ones64[:, 0:64], rod, start=st,
                            stop=sp, tile_position=(0, 64),
                            skip_group_check=True,
                        )

                B = bpool.tile([128, 512], F32, tag="B", name=f"B_{p}_{qc}")
                nc.vector.reciprocal_approx_fast(B[:], Z_ps[:])
                nc.vector.tensor_mul(
                    outT[:, p * N + qc * 512 : p * N + qc * 512 + 512],
                    O_ps[:],
                    B[:],
                )

        # ---------------- proj + store ------------------------------------
        for qt in range(8):
            ps = mm_psum(gi_box[0], [128, C])
            gi_box[0] += 1
            for p3 in range(3):
                nc.tensor.matmul(
                    ps[:],
                    outT[:, p3 * N + qt * 128 : p3 * N + (qt + 1) * 128],
                    wproj[:, p3 * C : (p3 + 1) * C],
                    start=(p3 == 0),
                    stop=(p3 == 2),
                )
            fin = bpool.tile([128, C], F32, tag="fin", name=f"fin_{qt}")
            if qt % 2 == 0:
                nc.scalar.copy(fin[:], ps[:])
            else:
                nc.vector.tensor_copy(fin[:], ps[:])
            nc.sync.dma_start(out_d[qt * 128 : (qt + 1) * 128, :], fin[:])

    nc.compile()
    return nc


def get_nc():
    if "nc" not in _CACHE:
        _CACHE["nc"] = _build()
    return _CACHE["nc"]


def make_in_maps(x, w_qkv, w_proj):
    wqkvT = np.ascontiguousarray(w_qkv.T).astype(np.float16)
    wprojT = np.ascontiguousarray(w_proj.T).astype(np.float16)
    return [
        {
            "xT": np.ascontiguousarray(x[b].T).astype(np.float16),
            "wqkvT": wqkvT,
            "wprojT": wprojT,
        }
        for b in range(x.shape[0])
    ]


def kernel(x, w_qkv, b_qkv, w_proj, b_proj):
    from concourse.bass_utils import run_bass_kernel_spmd

    x = np.asarray(x)
    assert x.shape == (NCORES, N, C), x.shape
    assert not np.asarray(b_qkv).any() and not np.asarray(b_proj).any(), (
        "kernel specialized for zero biases (problem spec fill=zeros)"
    )

    nc = get_nc()
    res = run_bass_kernel_spmd(nc, make_in_maps(x, w_qkv, w_proj), list(range(NCORES)))
    out = np.stack([res.results[i]["out"] for i in range(NCORES)], axis=0)
    return out.astype(np.float32)


if __name__ == "__main__":
    nc = get_nc()
    print("built + compiled OK:", nc)


# revision 10
# speedup vs baseline: 1.6539x; 1.6539x over previous
"""Trainium2 Bass kernel for nn_Attention_66314295050336.

Sparse (threshold-pruned) multi-head attention:
    qkv  = x @ w_qkv.T + b_qkv          [B,N,3C]
    q,k,v heads (H=6, D=64), attn = softmax(mask(q@k.T * D**-0.5))
    mask: scores < 0.0 -> -10000 before softmax (=> weight 0 in fp32)
    out  = (attn @ v) @ w_proj.T + b_proj

Sharding: pure data-parallel over batch B=8 across the 8 NeuronCores
(one batch per core, no collectives).  Per core everything stays
SBUF-resident; all matmuls are fp16 (1 cyc/row on the PE) with fp32
PSUM accumulation:

  xT[c,n] --(fp16 MM)--> qkT [768,1024] (heads paired on partitions)
                     +-> v   [1024,384] (fp16)
  scoresT[k,q] per head pair via row-packed K=64 matmuls
  ACT exp(scale*s) psum->SBUF fp16; threshold mask on DVE:
    b = (e >= 1)  [tensor_scalar, 4x]; e *= b [tensor_tensor, 2x]
  (e >= 1  <=>  score >= 0, measure-zero edge at exactly 0)
  attn@v: col-packed M=64 pairs, lhsT=v; a parallel ones-matmul gives
  Z (softmax denominator) replicated across 64 partitions, so
  normalization is reciprocal_approx_fast + one tensor_tensor multiply.
  proj: fp16 matmuls from SBUF, copied PSUM->SBUF, DMAed to HBM.

Biases are zeros per the problem spec (fill: zeros); asserted below.
"""

import os
import sys

import numpy as np

for _p in ("/opt/trn_rl_repo", "/root/.axon_site/_ro/trn_rl_repo"):
    if os.path.isdir(_p) and _p not in sys.path:
        sys.path.insert(0, _p)

N = 1024
C = 384
H = 6
D = 64
SCALE = float(D) ** -0.5  # 0.125
NCORES = 8

_CACHE = {}


def _build():
    import concourse.bass as bass
    import concourse.mybir as mybir
    import concourse.tile as tile
    from concourse import bacc
    from contextlib import ExitStack

    F32 = mybir.dt.float32
    F16 = mybir.dt.float16
    MULT = mybir.AluOpType.mult
    IS_GE = mybir.AluOpType.is_ge
    EXP = mybir.ActivationFunctionType.Exp

    nc = bacc.Bacc(
        "TRN2", target_bir_lowering=False, debug=False, enable_asserts=False
    )

    xT_d = nc.dram_tensor("xT", [C, N], F16, kind="ExternalInput")
    wqkvT_d = nc.dram_tensor("wqkvT", [C, 3 * C], F16, kind="ExternalInput")
    wprojT_d = nc.dram_tensor("wprojT", [C, C], F16, kind="ExternalInput")
    out_d = nc.dram_tensor("out", [N, C], F32, kind="ExternalOutput")

    with tile.TileContext(nc) as tc, ExitStack() as ctx:
        const = ctx.enter_context(tc.tile_pool(name="const", bufs=1))
        epool = ctx.enter_context(tc.tile_pool(name="e", bufs=2))
        bpool = ctx.enter_context(tc.tile_pool(name="bn", bufs=4))
        psS = ctx.enter_context(
            tc.tile_pool(name="psS", bufs=2, space=bass.MemorySpace.PSUM)
        )
        psO = ctx.enter_context(
            tc.tile_pool(name="psO", bufs=2, space=bass.MemorySpace.PSUM)
        )

        xT = const.tile([128, 3 * N], F16)  # c-tile ct -> cols [ct*N:(ct+1)*N]
        wqkv = const.tile([128, 3 * 3 * C], F16)  # ct -> cols [ct*1152 ...]
        wproj = const.tile([128, 3 * C], F16)
        qk = const.tile([128, 6 * N], F16)  # q pairs 0..2, k pairs 3..5
        v = const.tile([128, 8 * C], F16)  # n-tile nt -> cols [nt*384 ...]
        ones64 = const.tile([128, 64], F16)
        outT = const.tile([128, 3 * N], F16)  # pair p -> cols [p*N:(p+1)*N]

        for ct in range(3):
            r = slice(ct * 128, (ct + 1) * 128)
            nc.sync.dma_start(xT[:, ct * N : (ct + 1) * N], xT_d[r, :])
            nc.sync.dma_start(
                wqkv[:, ct * 1152 : (ct + 1) * 1152], wqkvT_d[r, :]
            )
            nc.sync.dma_start(wproj[:, ct * C : (ct + 1) * C], wprojT_d[r, :])
        nc.gpsimd.memset(ones64[:], 1.0)

        ps_tags = [("psS", "s"), ("psS", "s"), ("psO", "O"), ("psO", "Zb")]

        def mm_psum(pool_idx, shape):
            pn, tag = ps_tags[pool_idx % 4]
            pool = psS if pn == "psS" else psO
            return pool.tile(shape, F32, tag=tag, name=f"mmps_{pool_idx}")

        # ---------------- helpers: qkv-production groups ------------------
        gi_box = [0]

        def emit_qk_chunk(oc):
            # qkT o-chunk oc -> qk cols [oc*N : (oc+1)*N]
            for nh in range(2):
                ps = mm_psum(gi_box[0], [128, 512])
                gi_box[0] += 1
                for ct in range(3):
                    nc.tensor.matmul(
                        ps[:],
                        wqkv[
                            :, ct * 1152 + oc * 128 : ct * 1152 + (oc + 1) * 128
                        ],
                        xT[:, ct * N + nh * 512 : ct * N + nh * 512 + 512],
                        start=(ct == 0),
                        stop=(ct == 2),
                    )
                dst = qk[:, oc * N + nh * 512 : oc * N + nh * 512 + 512]
                if oc % 2 == 0:
                    nc.scalar.copy(dst, ps[:])
                else:
                    nc.vector.tensor_copy(dst, ps[:])

        # ---------------- v: [1024, 384] fp16 (n on partitions) -----------
        for nt in range(8):
            ps = mm_psum(gi_box[0], [128, 384])
            gi_box[0] += 1
            for ct in range(3):
                nc.tensor.matmul(
                    ps[:],
                    xT[:, ct * N + nt * 128 : ct * N + (nt + 1) * 128],
                    wqkv[:, ct * 1152 + 768 : ct * 1152 + 1152],
                    start=(ct == 0),
                    stop=(ct == 2),
                )
            nc.scalar.copy(v[:, nt * C : (nt + 1) * C], ps[:])

        emit_qk_chunk(0)  # q pair 0
        emit_qk_chunk(3)  # k pair 0

        # ---------------- attention, head pairs, qc-outer -----------------
        # e_pair layout: block bi=(qc*8+kt) -> cols [bi*1024 : bi*1024+1024],
        # block = [h_even 512 | h_odd 512] for that (kt, qc).
        for p in range(3):
            h_ev, h_od = 2 * p, 2 * p + 1
            e_pair = epool.tile([128, 16 * N], F16, tag="e", name=f"e_{p}")
            qT0 = p * N
            kT0 = (3 + p) * N
            if p < 2:  # produce next pair's qT/kT as PE fill-in work
                emit_qk_chunk(p + 1)
                emit_qk_chunk(3 + p + 1)

            for qc in range(2):
                O_ps = psO.tile([128, 512], F32, tag="O", name=f"O_{p}_{qc}")
                Z_ps = psO.tile([128, 512], F32, tag="Zb", name=f"Z_{p}_{qc}")
                for kt in range(8):
                    bi = qc * 8 + kt
                    s = psS.tile([128, 1024], F32, tag="s", name=f"s_{p}_{bi}")
                    # scoresT[k,q] = sum_d kT[d,k]*qT[d,q]; heads row-packed
                    nc.tensor.matmul(
                        s[:, 0:512],
                        qk[0:64, kT0 + kt * 128 : kT0 + (kt + 1) * 128],
                        qk[0:64, qT0 + qc * 512 : qT0 + (qc + 1) * 512],
                        start=True,
                        stop=True,
                    )
                    nc.tensor.matmul(
                        s[:, 512:1024],
                        qk[64:128, kT0 + kt * 128 : kT0 + (kt + 1) * 128],
                        qk[64:128, qT0 + qc * 512 : qT0 + (qc + 1) * 512],
                        start=True,
                        stop=True,
                    )
                    eb = e_pair[:, bi * 1024 : (bi + 1) * 1024]
                    nc.scalar.activation(eb, s[:], EXP, scale=SCALE)
                    if kt % 2 == 0:
                        continue
                    # threshold mask over two kt blocks at once:
                    # b = (e>=1) [TS 4x]; e *= b [TT 2x]
                    e2 = e_pair[:, (bi - 1) * 1024 : (bi + 1) * 1024]
                    b = bpool.tile(
                        [128, 2048], F16, tag="b", name=f"b_{p}_{bi}"
                    )
                    nc.vector.tensor_scalar(b[:], e2, 1.0, None, IS_GE)
                    nc.vector.tensor_mul(e2, e2, b[:])
                    for kb in (kt - 1, kt):
                        bj = qc * 8 + kb
                        rev = e_pair[:, bj * 1024 : bj * 1024 + 512]
                        rod = e_pair[:, bj * 1024 + 512 : bj * 1024 + 1024]
                        vev = v[:, kb * C + h_ev * 64 : kb * C + h_ev * 64 + 64]
                        vod = v[:, kb * C + h_od * 64 : kb * C + h_od * 64 + 64]
                        st, sp = (kb == 0), (kb == 7)
                        # outT_h[d,q] accumulation, two heads col-packed
                        nc.tensor.matmul(
                            O_ps[0:64, :], vev, rev, start=st, stop=sp,
                            tile_position=(0, 0), skip_group_check=True,
                        )
                        nc.tensor.matmul(
                            O_ps[64:128, :], vod, rod, start=st, stop=sp,
                            tile_position=(0, 64), skip_group_check=True,
                        )
                        # Z_h[q] (replicated x64): ones-matmul, same rhs
                        nc.tensor.matmul(
                            Z_ps[0:64, :], ones64[:, 0:64], rev, start=st,
                            stop=sp, tile_position=(0, 0),
                            skip_group_check=True,
                        )
                        nc.tensor.matmul(
                            Z_ps[64:128, :], ones64[:, 0:64], rod, start=st,
                            stop=sp, tile_position=(0, 64),
                            skip_group_check=True,
                        )

                B = bpool.tile([128, 512], F32, tag="B", name=f"B_{p}_{qc}")
                nc.vector.reciprocal_approx_fast(B[:], Z_ps[:])
                nc.vector.tensor_mul(
                    outT[:, p * N + qc * 512 : p * N + qc * 512 + 512],
                    O_ps[:],
                    B[:],
                )

        # ---------------- proj + store ------------------------------------
        for qt in range(8):
            ps = mm_psum(gi_box[0], [128, C])
            gi_box[0] += 1
            for p3 in range(3):
                nc.tensor.matmul(
                    ps[:],
                    outT[:, p3 * N + qt * 128 : p3 * N + (qt + 1) * 128],
                    wproj[:, p3 * C : (p3 + 1) * C],
                    start=(p3 == 0),
                    stop=(p3 == 2),
                )
            fin = bpool.tile([128, C], F32, tag="fin", name=f"fin_{qt}")
            if qt % 2 == 0:
                nc.scalar.copy(fin[:], ps[:])
            else:
                nc.vector.tensor_copy(fin[:], ps[:])
            nc.sync.dma_start(out_d[qt * 128 : (qt + 1) * 128, :], fin[:])

    nc.compile()
    return nc


def get_nc():
    if "nc" not in _CACHE:
        _CACHE["nc"] = _build()
    return _CACHE["nc"]


def make_in_maps(x, w_qkv, w_proj):
    wqkvT = np.ascontiguousarray(w_qkv.T).astype(np.float16)
    wprojT = np.ascontiguousarray(w_proj.T).astype(np.float16)
    return [
        {
            "xT": np.ascontiguousarray(x[b].T).astype(np.float16),
            "wqkvT": wqkvT,
            "wprojT": wprojT,
        }
        for b in range(x.shape[0])
    ]


def kernel(x, w_qkv, b_qkv, w_proj, b_proj):
    from concourse.bass_utils import run_bass_kernel_spmd

    x = np.asarray(x)
    assert x.shape == (NCORES, N, C), x.shape
    assert not np.asarray(b_qkv).any() and not np.asarray(b_proj).any(), (
        "kernel specialized for zero biases (problem spec fill=zeros)"
    )

    nc = get_nc()
    res = run_bass_kernel_spmd(nc, make_in_maps(x, w_qkv, w_proj), list(range(NCORES)))
    out = np.stack([res.results[i]["out"] for i in range(NCORES)], axis=0)
    return out.astype(np.float32)


if __name__ == "__main__":
    nc = get_nc()
    print("built + compiled OK:", nc)
